# revision 30
# baseline (speedup 1.0000x reference)
"""Trainium2 Bass kernel for nn_Controller (batch-1 two-layer LSTM-cell chain
+ choice head), distributed over 8 NeuronCores.

Math notes (from the module semantics): both LSTMCells run with zero initial
state, so the h @ W_hh.T terms are identically zero and the f-gate multiplies
c=0.  Only the i/g/o thirds of each W_ih are ever needed:
    gates = x @ W_ih.T + (b_ih + b_hh)
    h     = sigmoid(o) * tanh(sigmoid(i) * tanh(g))
That cuts required HBM traffic from 256 MiB to 96 MiB before sharding.

Distribution (v2): layer 0 is row-sharded (each core computes its 256-slot
h0 chunk from 768 gate rows); layer 1 is COLUMN-sharded: each core holds
W1[:, its 256 h0 columns] for all 6144 i/g/o rows and computes a partial
gate pre-activation [6144].  One bf16 AllReduce sums the partials; every
core then applies the bias + activations and the choice head locally, so
the program has exactly ONE collective (the CC-stream setup cost scales
with collective count, and the h0 exchange disappears entirely).

GEMVs run x-stationary on the PE (stationary = one 128-row x column,
moving = 384..512-wide weight slabs), which replaces the per-matmul
128-cycle weight load + 173ns instruction latency of the W-stationary
form with ~1/3 the instruction count.  All sigmoids are computed via
sigmoid(x) = (1 + tanh(x/2))/2 so the scalar engine's activation table is
loaded exactly once (Tanh); the residual *2 factors are folded into the
host-side scaling of W1 and W_choice.
"""

import os
import sys

import numpy as np
import ml_dtypes

for _p in ("/opt/trn_rl_repo", os.path.expanduser("~/.axon_site/_ro/trn_rl_repo")):
    if os.path.isdir(_p) and _p not in sys.path:
        sys.path.insert(0, _p)

import concourse.bass as bass
import concourse.bacc as bacc
import concourse.mybir as mybir
import concourse.tile as tile
from concourse.bass_utils import run_bass_kernel_spmd

H = 2048
NCORES = 8
C = H // NCORES          # 256: per-core h chunk
NK = H // 128            # 16 k-tiles
G1 = 4 * H - H           # 6144 needed gate rows (i/g/o)
NG = 16                  # layer-1 partial groups (384 wide each)
GW = G1 // NG            # 384
CH = 19                  # choice logits
DT = mybir.dt.float32
DTW = mybir.dt.bfloat16  # weight/activation-stream dtype
BF = ml_dtypes.bfloat16
Act = mybir.ActivationFunctionType


# --------------------------------------------------------------------------
# host-side layout prep
# --------------------------------------------------------------------------

def _r0(k):
    """Layer-0 gate rows handled by core k, in [i | o | g] order along the
    768-wide output axis (i,o first so one Tanh op covers both)."""
    a = np.arange(C)
    return np.concatenate([0 * H + k * C + a, 3 * H + k * C + a, 2 * H + k * C + a])


def _r1():
    """Layer-1 global row order: flat index m = q*48 + t reads back as a
    [128, 48] tile with cols [i(16) | o(16) | g(16)] and h-index q*16+u."""
    m = np.arange(G1)
    q, t = m // 48, m % 48
    base = np.where(t < 16, 0, np.where(t < 32, 3 * H, 2 * H))
    u = np.where(t < 16, t, np.where(t < 32, t - 16, t - 32))
    return base + q * 16 + u


def _host_prep(inputs):
    idx = int(np.asarray(inputs["input_idx"]).reshape(-1)[0])
    emb = np.asarray(inputs["embedding"], np.float32)
    x0T = np.ascontiguousarray(emb[idx].reshape(NK, 128).T.astype(BF))

    W0 = np.asarray(inputs["w_ih_0"], np.float32)
    W1 = np.asarray(inputs["w_ih_1"], np.float32)
    B0 = np.asarray(inputs["b_ih_0"], np.float32) + np.asarray(inputs["b_hh_0"], np.float32)
    B1 = np.asarray(inputs["b_ih_1"], np.float32) + np.asarray(inputs["b_hh_1"], np.float32)
    WC = np.asarray(inputs["w_choice"], np.float32)
    BC = np.asarray(inputs["b_choice"], np.float32)

    r1 = _r1()
    # g-gate rows (cols 32:48 of the [128,48] layout) carry an extra x2 so the
    # whole post-AllReduce tile goes through ONE Tanh with scale=0.5:
    # tanh(i/2), tanh(o/2), tanh(g) in a single activation op
    gmul = np.concatenate([np.ones(32), np.full(16, 2.0)])[None, :].repeat(128, 0).reshape(-1)
    # B1/8 per core, folded into the partial-gate drains so the bias rides the
    # AllGather and the post-collective path needs no separate bias add
    b1h = np.ascontiguousarray((B1[r1] * gmul / NCORES).reshape(1, G1).astype(BF))
    # wch[q, u*19+j] = 0.5*WC[j, q*16+u]
    wch = np.ascontiguousarray(
        0.5 * np.transpose(WC.reshape(CH, 128, 16), (1, 2, 0)).reshape(128, 16 * CH)
    ).astype(BF)
    bch = np.ascontiguousarray(BC.reshape(1, CH))
    onesh = np.ones((1, 1), BF)

    maps = []
    for k in range(NCORES):
        R = _r0(k)
        # w0h[t, p, m] = W0[r0[m], t*128+p]
        w0h = np.ascontiguousarray(W0[R].T.reshape(NK, 128, 3 * C).astype(BF))
        b0h = np.ascontiguousarray(B0[R].reshape(1, 3 * C).astype(BF))
        # w1h[j, i, p, c] = 0.5*gmul*W1[r1[i*768+c], k*256 + j*128 + p]
        Ws = (0.5 * gmul)[:, None] * W1[r1][:, k * C:(k + 1) * C]  # [6144, 256]
        w1h = np.ascontiguousarray(
            np.transpose(Ws.reshape(G1, 2, 128), (1, 2, 0))  # [2, 128, 6144]
            .reshape(2, 128, 8, 768).transpose(0, 2, 1, 3)   # [2, 8, 128, 768]
        ).astype(BF)
        maps.append(dict(x0T=x0T, w0=w0h, b0=b0h, w1=w1h, b1=b1h,
                         wc=wch, bc=bch, ones=onesh))
    return maps


# --------------------------------------------------------------------------
# device program (identical on all 8 cores; per-core data differs)
# --------------------------------------------------------------------------

def _build_nc():
    nc = bacc.Bacc("TRN2", target_bir_lowering=False, debug=False,
                   num_devices=NCORES)

    x0T = nc.dram_tensor("x0T", [128, NK], DTW, kind="ExternalInput")
    w0 = nc.dram_tensor("w0", [NK, 128, 3 * C], DTW, kind="ExternalInput")
    b0 = nc.dram_tensor("b0", [1, 3 * C], DTW, kind="ExternalInput")
    w1 = nc.dram_tensor("w1", [2, 8, 128, 768], DTW, kind="ExternalInput")
    b1 = nc.dram_tensor("b1", [1, G1], DTW, kind="ExternalInput")
    wc = nc.dram_tensor("wc", [128, 16 * CH], DTW, kind="ExternalInput")
    bc = nc.dram_tensor("bc", [1, CH], DT, kind="ExternalInput")
    ones = nc.dram_tensor("ones", [1, 1], DTW, kind="ExternalInput")
    out = nc.dram_tensor("out", [CH], DT, kind="ExternalOutput")
    # Shared address space: the HBM-HBM collective fast path
    cc_out = nc.dram_tensor("cc_out", [NCORES * G1], DTW, kind="Internal",
                            addr_space="Shared")

    rg = [list(range(NCORES))]

    with tile.TileContext(nc) as tc:
        with (
            tc.tile_pool(name="weights", bufs=1) as wp,
            tc.tile_pool(name="small", bufs=1) as sp,
            tc.tile_pool(name="act", bufs=1) as ap,
            tc.tile_pool(name="psum", bufs=1, space=bass.MemorySpace.PSUM) as pp,
            tc.tile_pool(name="dram", bufs=1, space=bass.MemorySpace.DRAM) as dp,
        ):
            # small loads on gpsimd (SWDGE); weight stream owns the sync queue
            onesb = sp.tile([1, 1], DTW, tag="ones")
            nc.gpsimd.dma_start(onesb[:], ones[:])
            x0sb = sp.tile([128, NK], DTW, tag="x0")
            nc.gpsimd.dma_start(x0sb[:], x0T[:])
            b0sb = sp.tile([1, 3 * C], DTW, tag="b0")
            nc.gpsimd.dma_start(b0sb[:], b0[:])
            b1sb = sp.tile([1, G1], DTW, tag="b1")
            nc.gpsimd.dma_start(b1sb[:], b1[:])
            wcsb = sp.tile([128, 16 * CH], DTW, tag="wc")
            nc.gpsimd.dma_start(wcsb[:], wc[:])
            bcsb = sp.tile([1, CH], DT, tag="bc")
            nc.gpsimd.dma_start(bcsb[:], bc[:])

            # load the Tanh activation table once, early
            warm = ap.tile([1, 1], DT, tag="warm")
            nc.scalar.activation(warm[:], onesb[:], Act.Tanh)
            # fp32 copy of the ones tile: identity for the fp32 PE transposes
            ones32 = ap.tile([1, 1], DT, tag="ones32")
            nc.vector.tensor_copy(ones32[:], onesb[:])

            # weight stream (sync HWDGE, 16 DMA engines round-robin)
            w0sb = []
            for t in range(NK):
                wt = wp.tile([128, 3 * C], DTW, tag=f"w0_{t}")
                nc.sync.dma_start(wt[:], w0[t])
                w0sb.append(wt)
            w1sb = []
            for j in range(2):
                for i in range(8):
                    wt = wp.tile([128, 768], DTW, tag=f"w1_{j}_{i}")
                    nc.sync.dma_start(wt[:], w1[j, i])
                    w1sb.append(wt)

            # ---- layer 0: x-stationary GEMV, 768 rows = [i|o|g] ----
            psA = pp.tile([1, 512], DT, tag="psA")
            psB = pp.tile([1, 256], DT, tag="psB")
            nc.tensor.matmul(psA[:], onesb[:], b0sb[:, 0:512], start=True, stop=False)
            nc.tensor.matmul(psB[:], onesb[:], b0sb[:, 512:768], start=True, stop=False)
            for t in range(NK):
                nc.tensor.matmul(psA[:], x0sb[:, t:t + 1], w0sb[t][:, 0:512],
                                 start=False, stop=(t == NK - 1))
                nc.tensor.matmul(psB[:], x0sb[:, t:t + 1], w0sb[t][:, 512:768],
                                 start=False, stop=(t == NK - 1))

            # activations: h' = 2h = tanh(c)*(1+tanh(o/2)), c = tanh(g)*(1+tanh(i/2))/1
            t_io0 = ap.tile([1, 512], DT, tag="t_io0")
            nc.scalar.activation(t_io0[:], psA[:], Act.Tanh, scale=0.5)
            t_g0 = ap.tile([1, 256], DT, tag="t_g0")
            nc.scalar.activation(t_g0[:], psB[:], Act.Tanh)
            tmp0 = ap.tile([1, 256], DT, tag="tmp0")
            nc.vector.tensor_mul(tmp0[:], t_g0[:], t_io0[:, 0:256])
            c2 = ap.tile([1, 256], DT, tag="c2")
            nc.vector.tensor_add(c2[:], tmp0[:], t_g0[:])
            t_c0 = ap.tile([1, 256], DT, tag="t_c0")
            nc.scalar.activation(t_c0[:], c2[:], Act.Tanh, scale=0.5)
            tmp1 = ap.tile([1, 256], DT, tag="tmp1")
            nc.vector.tensor_mul(tmp1[:], t_c0[:], t_io0[:, 256:512])
            h0row = ap.tile([1, 256], DT, tag="h0row")
            nc.vector.tensor_add(h0row[:], tmp1[:], t_c0[:])

            # transpose h' [1,256] -> [128,2] for the layer-1 stationary x
            # (fp32 transpose: psum writes must stay 4-byte aligned)
            psT = pp.tile([128, 2], DT, tag="psT")
            nc.tensor.transpose(psT[:, 0:1], h0row[0:1, 0:128], ones32[:])
            nc.tensor.transpose(psT[:, 1:2], h0row[0:1, 128:256], ones32[:])
            h0T = ap.tile([128, 2], DTW, tag="h0T")
            nc.vector.tensor_copy(h0T[:], psT[:])

            # ---- layer 1: column-sharded partial gates, 16 groups of 384.
            # B1/8 enters each group's psum via a ones-row matmul (hidden under
            # the weight-DMA chase), so the drains are pure copies and split
            # across the vector and scalar engines in parallel ----
            g1part = sp.tile([1, G1], DTW, tag="g1part")
            cc_in = dp.tile([G1], DTW, tag="cc_in")
            cc_in_v = cc_in.rearrange("(a n) -> a n", a=1)
            Q = G1 // 4
            for g in range(NG):
                ps = pp.tile([1, GW], DT, tag=f"g{g % 4}", name=f"ps_g{g}")
                i, c0 = g // 2, (g % 2) * GW
                nc.tensor.matmul(ps[:], onesb[:], b1sb[:, g * GW:(g + 1) * GW],
                                 start=True, stop=False)
                nc.tensor.matmul(ps[:], h0T[:, 0:1], w1sb[i][:, c0:c0 + GW],
                                 start=False, stop=False)
                nc.tensor.matmul(ps[:], h0T[:, 1:2], w1sb[8 + i][:, c0:c0 + GW],
                                 start=False, stop=True)
                dst = g1part[:, g * GW:(g + 1) * GW]
                if g % 2:
                    nc.scalar.activation(dst, ps[:], Act.Copy)
                else:
                    nc.vector.tensor_copy(dst, ps[:])
                if g % 4 == 3:  # stage the finished quarter while later groups run
                    q = g // 4
                    nc.sync.dma_start(cc_in_v[:, q * Q:(q + 1) * Q],
                                      g1part[:, q * Q:(q + 1) * Q])

            # ---- the single collective: AllGather of the partial gates ----
            # (cheaper as a first CC op than AllReduce; the 8-way sum runs on
            # the vector engine in fp32, which is also more accurate than the
            # CC cores' bf16 tree).  The input was staged in 4 chunks above,
            # pipelined behind the drains.
            nc.gpsimd.collective_compute(
                "AllGather", mybir.AluOpType.bypass,
                ins=[cc_in.opt()], outs=[cc_out.rearrange("(n) -> n").opt()],
                replica_groups=rg,
            )
            # readback rank-outer (96B contiguous runs per rank chunk), split
            # across both HWDGE queues (sync + scalar) to overlap dispatch
            g8 = sp.tile([128, NCORES * 48], DTW, tag="g8")
            cc_v = cc_out.rearrange("(r q t) -> q r t", r=NCORES, t=48)
            nc.sync.dma_start(g8[:, 0:192], cc_v[:, 0:4])
            nc.scalar.dma_start(g8[:, 192:384], cc_v[:, 4:8])

            # ---- post-AllGather: 8-way tree sum, contiguous reads ----
            h4 = ap.tile([128, 4 * 48], DT, tag="h4")
            nc.vector.tensor_add(h4[:], g8[:, 0:192], g8[:, 192:384])
            h2 = ap.tile([128, 2 * 48], DT, tag="h2")
            nc.vector.tensor_add(h2[:], h4[:, 0:96], h4[:, 96:192])
            gsum = ap.tile([128, 48], DT, tag="gsum")
            nc.vector.tensor_add(gsum[:], h2[:, 0:48], h2[:, 48:96])
            t_all = ap.tile([128, 48], DT, tag="t_all")
            nc.scalar.activation(t_all[:], gsum[:], Act.Tanh, scale=0.5)
            t_io1 = t_all  # cols 0:16 = tanh(i/2), 16:32 = tanh(o/2)
            t_g1 = t_all[:, 32:48]  # g rows were pre-doubled: tanh(2g/2) = tanh(g)
            tmpa = ap.tile([128, 16], DT, tag="tmpa")
            nc.vector.tensor_mul(tmpa[:], t_g1, t_io1[:, 0:16])
            c21 = ap.tile([128, 16], DT, tag="c21")
            nc.vector.tensor_add(c21[:], tmpa[:], t_g1)
            t_c1 = ap.tile([128, 16], DT, tag="t_c1")
            nc.scalar.activation(t_c1[:], c21[:], Act.Tanh, scale=0.5)
            tmpb = ap.tile([128, 16], DT, tag="tmpb")
            nc.vector.tensor_mul(tmpb[:], t_c1[:], t_io1[:, 16:32])
            h1 = ap.tile([128, 16], DTW, tag="h1")
            nc.vector.tensor_add(h1[:], tmpb[:], t_c1[:])

            # ---- choice head: logits = 0.5*WC @ h1' + bc, computed locally ----
            psH = pp.tile([1, CH], DT, tag="psH")
            for u in range(16):
                nc.tensor.matmul(psH[:], h1[:, u:u + 1], wcsb[:, u * CH:(u + 1) * CH],
                                 start=(u == 0), stop=(u == 15))
            outsb = ap.tile([1, CH], DT, tag="outsb")
            nc.vector.tensor_add(outsb[:], psH[:], bcsb[:])
            nc.sync.dma_start(out.rearrange("(a n) -> a n", a=1), outsb[:])

    nc.compile()
    return nc


_NC_CACHE = None


def _get_nc():
    global _NC_CACHE
    if _NC_CACHE is None:
        _NC_CACHE = _build_nc()
    return _NC_CACHE


# --------------------------------------------------------------------------
# entry point
# --------------------------------------------------------------------------

def kernel(**inputs) -> np.ndarray:
    task = int(np.asarray(inputs["task"]).reshape(-1)[0]) if not isinstance(
        inputs["task"], int) else int(inputs["task"])
    maps = _host_prep(inputs)
    nc = _get_nc()
    for attempt in range(3):
        res = run_bass_kernel_spmd(nc, maps, list(range(NCORES)))
        outs = [np.asarray(res.results[i]["out"], np.float32).reshape(CH)
                for i in range(NCORES)]
        # post-AllReduce every core holds the same logits (up to reduction
        #-order LSBs); gross disagreement means a bad device state -- retry
        if all(np.allclose(outs[0], o, atol=1e-3) for o in outs[1:]):
            break
    logits = outs[0]
    mask = np.arange(CH) < (1 + task)
    return np.where(mask, logits, np.float32(-1e9)).astype(np.float32)


if __name__ == "__main__":
    import reference  # only for standalone debugging; not used by the grader

    inputs = reference.setup_inputs()
    expected = np.asarray(reference.reference(**inputs))
    actual = kernel(**inputs)
    print("expected:", expected)
    print("actual:  ", actual)
    denom = np.abs(expected).max()
    print("max abs err:", np.abs(actual - expected).max(),
          "rel:", np.abs(actual - expected).max() / denom)


# revision 32
# speedup vs baseline: 1.2162x; 1.2162x over previous
"""Trainium2 Bass kernel for nn_Controller (batch-1 two-layer LSTM-cell chain
+ choice head), distributed over 8 NeuronCores.

Math notes (from the module semantics): both LSTMCells run with zero initial
state, so the h @ W_hh.T terms are identically zero and the f-gate multiplies
c=0.  Only the i/g/o thirds of each W_ih are ever needed:
    gates = x @ W_ih.T + (b_ih + b_hh)
    h     = sigmoid(o) * tanh(sigmoid(i) * tanh(g))
That cuts required HBM traffic from 256 MiB to 96 MiB before sharding.

Distribution (v2): layer 0 is row-sharded (each core computes its 256-slot
h0 chunk from 768 gate rows); layer 1 is COLUMN-sharded: each core holds
W1[:, its 256 h0 columns] for all 6144 i/g/o rows and computes a partial
gate pre-activation [6144].  One bf16 AllReduce sums the partials; every
core then applies the bias + activations and the choice head locally, so
the program has exactly ONE collective (the CC-stream setup cost scales
with collective count, and the h0 exchange disappears entirely).

GEMVs run x-stationary on the PE (stationary = one 128-row x column,
moving = 384..512-wide weight slabs), which replaces the per-matmul
128-cycle weight load + 173ns instruction latency of the W-stationary
form with ~1/3 the instruction count.  All sigmoids are computed via
sigmoid(x) = (1 + tanh(x/2))/2 so the scalar engine's activation table is
loaded exactly once (Tanh); the residual *2 factors are folded into the
host-side scaling of W1 and W_choice.
"""

import os
import sys

import numpy as np
import ml_dtypes

for _p in ("/opt/trn_rl_repo", os.path.expanduser("~/.axon_site/_ro/trn_rl_repo")):
    if os.path.isdir(_p) and _p not in sys.path:
        sys.path.insert(0, _p)

import concourse.bass as bass
import concourse.bacc as bacc
import concourse.mybir as mybir
import concourse.tile as tile
from concourse.bass_utils import run_bass_kernel_spmd

H = 2048
NCORES = 8
C = H // NCORES          # 256: per-core h chunk
NK = H // 128            # 16 k-tiles
G1 = 4 * H - H           # 6144 needed gate rows (i/g/o)
NG = 16                  # layer-1 partial groups (384 wide each)
GW = G1 // NG            # 384
CH = 19                  # choice logits
DT = mybir.dt.float32
DTW = mybir.dt.bfloat16  # weight/activation-stream dtype
BF = ml_dtypes.bfloat16
Act = mybir.ActivationFunctionType


# --------------------------------------------------------------------------
# host-side layout prep
# --------------------------------------------------------------------------

def _r0(k):
    """Layer-0 gate rows handled by core k, in [i | o | g] order along the
    768-wide output axis (i,o first so one Tanh op covers both)."""
    a = np.arange(C)
    return np.concatenate([0 * H + k * C + a, 3 * H + k * C + a, 2 * H + k * C + a])


def _r1():
    """Layer-1 global row order: flat index m = q*48 + t reads back as a
    [128, 48] tile with cols [i(16) | o(16) | g(16)] and h-index q*16+u."""
    m = np.arange(G1)
    q, t = m // 48, m % 48
    base = np.where(t < 16, 0, np.where(t < 32, 3 * H, 2 * H))
    u = np.where(t < 16, t, np.where(t < 32, t - 16, t - 32))
    return base + q * 16 + u


def _host_prep(inputs):
    idx = int(np.asarray(inputs["input_idx"]).reshape(-1)[0])
    emb = np.asarray(inputs["embedding"], np.float32)
    x0T = np.ascontiguousarray(emb[idx].reshape(NK, 128).T.astype(BF))

    W0 = np.asarray(inputs["w_ih_0"], np.float32)
    W1 = np.asarray(inputs["w_ih_1"], np.float32)
    B0 = np.asarray(inputs["b_ih_0"], np.float32) + np.asarray(inputs["b_hh_0"], np.float32)
    B1 = np.asarray(inputs["b_ih_1"], np.float32) + np.asarray(inputs["b_hh_1"], np.float32)
    WC = np.asarray(inputs["w_choice"], np.float32)
    BC = np.asarray(inputs["b_choice"], np.float32)

    r1 = _r1()
    # g-gate rows (cols 32:48 of the [128,48] layout) carry an extra x2 so the
    # whole post-AllReduce tile goes through ONE Tanh with scale=0.5:
    # tanh(i/2), tanh(o/2), tanh(g) in a single activation op
    gmul = np.concatenate([np.ones(32), np.full(16, 2.0)])[None, :].repeat(128, 0).reshape(-1)
    # B1/8 per core, folded into the partial-gate drains so the bias rides the
    # AllGather and the post-collective path needs no separate bias add
    b1h = np.ascontiguousarray((B1[r1] * gmul / NCORES).reshape(1, G1).astype(BF))
    # wch[q, u*19+j] = 0.5*WC[j, q*16+u]
    wch = np.ascontiguousarray(
        0.5 * np.transpose(WC.reshape(CH, 128, 16), (1, 2, 0)).reshape(128, 16 * CH)
    ).astype(BF)
    bch = np.ascontiguousarray(BC.reshape(1, CH))
    onesh = np.ones((1, 1), BF)

    maps = []
    for k in range(NCORES):
        R = _r0(k)
        # w0h[t, p, m] = W0[r0[m], t*128+p], packed pairwise into 3KB lines:
        # w0p[i, p, :] = [k-tile 2i | k-tile 2i+1]
        w0h = W0[R].T.reshape(NK, 128, 3 * C).astype(BF)
        w0p = np.ascontiguousarray(
            w0h.reshape(8, 2, 128, 3 * C).transpose(0, 2, 1, 3).reshape(8, 128, 2 * 3 * C))
        b0h = np.ascontiguousarray(B0[R].reshape(1, 3 * C).astype(BF))
        # w1h[j, i, p, c] = 0.5*gmul*W1[r1[i*1536+c], k*256 + j*128 + p]
        Ws = (0.5 * gmul)[:, None] * W1[r1][:, k * C:(k + 1) * C]  # [6144, 256]
        w1h = np.ascontiguousarray(
            np.transpose(Ws.reshape(G1, 2, 128), (1, 2, 0))  # [2, 128, 6144]
            .reshape(2, 128, 4, 1536).transpose(0, 2, 1, 3)  # [2, 4, 128, 1536]
        ).astype(BF)
        maps.append(dict(x0T=x0T, w0=w0p, b0=b0h, w1=w1h, b1=b1h,
                         wc=wch, bc=bch, ones=onesh))
    return maps


# --------------------------------------------------------------------------
# device program (identical on all 8 cores; per-core data differs)
# --------------------------------------------------------------------------

def _build_nc():
    nc = bacc.Bacc("TRN2", target_bir_lowering=False, debug=False,
                   num_devices=NCORES)

    x0T = nc.dram_tensor("x0T", [128, NK], DTW, kind="ExternalInput")
    w0 = nc.dram_tensor("w0", [8, 128, 1536], DTW, kind="ExternalInput")
    b0 = nc.dram_tensor("b0", [1, 3 * C], DTW, kind="ExternalInput")
    w1 = nc.dram_tensor("w1", [2, 4, 128, 1536], DTW, kind="ExternalInput")
    b1 = nc.dram_tensor("b1", [1, G1], DTW, kind="ExternalInput")
    wc = nc.dram_tensor("wc", [128, 16 * CH], DTW, kind="ExternalInput")
    bc = nc.dram_tensor("bc", [1, CH], DT, kind="ExternalInput")
    ones = nc.dram_tensor("ones", [1, 1], DTW, kind="ExternalInput")
    out = nc.dram_tensor("out", [CH], DT, kind="ExternalOutput")
    # Shared address space: the HBM-HBM collective fast path
    cc_out = nc.dram_tensor("cc_out", [NCORES * G1], DTW, kind="Internal",
                            addr_space="Shared")

    rg = [list(range(NCORES))]

    with tile.TileContext(nc) as tc:
        with (
            tc.tile_pool(name="weights", bufs=1) as wp,
            tc.tile_pool(name="small", bufs=1) as sp,
            tc.tile_pool(name="act", bufs=1) as ap,
            tc.tile_pool(name="psum", bufs=1, space=bass.MemorySpace.PSUM) as pp,
            tc.tile_pool(name="dram", bufs=1, space=bass.MemorySpace.DRAM) as dp,
        ):
            # small loads on gpsimd (SWDGE); weight stream owns the sync queue
            onesb = sp.tile([1, 1], DTW, tag="ones")
            nc.gpsimd.dma_start(onesb[:], ones[:])
            x0sb = sp.tile([128, NK], DTW, tag="x0")
            nc.gpsimd.dma_start(x0sb[:], x0T[:])
            b0sb = sp.tile([1, 3 * C], DTW, tag="b0")
            nc.gpsimd.dma_start(b0sb[:], b0[:])
            b1sb = sp.tile([1, G1], DTW, tag="b1")
            nc.gpsimd.dma_start(b1sb[:], b1[:])
            wcsb = sp.tile([128, 16 * CH], DTW, tag="wc")
            nc.gpsimd.dma_start(wcsb[:], wc[:])
            bcsb = sp.tile([1, CH], DT, tag="bc")
            nc.gpsimd.dma_start(bcsb[:], bc[:])

            # load the Tanh activation table once, early
            warm = ap.tile([1, 1], DT, tag="warm")
            nc.scalar.activation(warm[:], onesb[:], Act.Tanh)
            # fp32 copy of the ones tile: identity for the fp32 PE transposes
            ones32 = ap.tile([1, 1], DT, tag="ones32")
            nc.vector.tensor_copy(ones32[:], onesb[:])

            # weight stream (sync HWDGE): 3KB lines, each [128,1536] tile
            # split into two 64-partition DMAs so all 16 engines stay busy
            w0sb = []
            for t in range(8):
                wt = wp.tile([128, 1536], DTW, tag=f"w0_{t}")
                nc.sync.dma_start(wt[0:64, :], w0[t, 0:64])
                nc.sync.dma_start(wt[64:128, :], w0[t, 64:128])
                w0sb.append(wt)
            w1sb = []
            for j in range(2):
                for i in range(4):
                    wt = wp.tile([128, 1536], DTW, tag=f"w1_{j}_{i}")
                    nc.sync.dma_start(wt[0:64, :], w1[j, i, 0:64])
                    nc.sync.dma_start(wt[64:128, :], w1[j, i, 64:128])
                    w1sb.append(wt)

            # ---- layer 0: x-stationary GEMV, 768 rows = [i|o|g] ----
            psA = pp.tile([1, 512], DT, tag="psA")
            psB = pp.tile([1, 256], DT, tag="psB")
            nc.tensor.matmul(psA[:], onesb[:], b0sb[:, 0:512], start=True, stop=False)
            nc.tensor.matmul(psB[:], onesb[:], b0sb[:, 512:768], start=True, stop=False)
            for t in range(NK):
                wt, base = w0sb[t // 2], (t % 2) * 768
                nc.tensor.matmul(psA[:], x0sb[:, t:t + 1], wt[:, base:base + 512],
                                 start=False, stop=(t == NK - 1))
                nc.tensor.matmul(psB[:], x0sb[:, t:t + 1], wt[:, base + 512:base + 768],
                                 start=False, stop=(t == NK - 1))

            # activations: h' = 2h = tanh(c)*(1+tanh(o/2)), c = tanh(g)*(1+tanh(i/2))/1
            t_io0 = ap.tile([1, 512], DT, tag="t_io0")
            nc.scalar.activation(t_io0[:], psA[:], Act.Tanh, scale=0.5)
            t_g0 = ap.tile([1, 256], DT, tag="t_g0")
            nc.scalar.activation(t_g0[:], psB[:], Act.Tanh)
            tmp0 = ap.tile([1, 256], DT, tag="tmp0")
            nc.vector.tensor_mul(tmp0[:], t_g0[:], t_io0[:, 0:256])
            c2 = ap.tile([1, 256], DT, tag="c2")
            nc.vector.tensor_add(c2[:], tmp0[:], t_g0[:])
            t_c0 = ap.tile([1, 256], DT, tag="t_c0")
            nc.scalar.activation(t_c0[:], c2[:], Act.Tanh, scale=0.5)
            tmp1 = ap.tile([1, 256], DT, tag="tmp1")
            nc.vector.tensor_mul(tmp1[:], t_c0[:], t_io0[:, 256:512])
            h0row = ap.tile([1, 256], DT, tag="h0row")
            nc.vector.tensor_add(h0row[:], tmp1[:], t_c0[:])

            # transpose h' [1,256] -> [128,2] for the layer-1 stationary x
            # (fp32 transpose: psum writes must stay 4-byte aligned)
            psT = pp.tile([128, 2], DT, tag="psT")
            nc.tensor.transpose(psT[:, 0:1], h0row[0:1, 0:128], ones32[:])
            nc.tensor.transpose(psT[:, 1:2], h0row[0:1, 128:256], ones32[:])
            h0T = ap.tile([128, 2], DTW, tag="h0T")
            nc.vector.tensor_copy(h0T[:], psT[:])

            # ---- layer 1: column-sharded partial gates, 16 groups of 384.
            # B1/8 enters each group's psum via a ones-row matmul (hidden under
            # the weight-DMA chase), so the drains are pure copies and split
            # across the vector and scalar engines in parallel ----
            g1part = sp.tile([1, G1], DTW, tag="g1part")
            cc_in = dp.tile([G1], DTW, tag="cc_in")
            cc_in_v = cc_in.rearrange("(a n) -> a n", a=1)
            Q = G1 // 4
            for g in range(NG):
                ps = pp.tile([1, GW], DT, tag=f"g{g % 4}", name=f"ps_g{g}")
                i, c0 = g // 4, (g % 4) * GW
                nc.tensor.matmul(ps[:], onesb[:], b1sb[:, g * GW:(g + 1) * GW],
                                 start=True, stop=False)
                nc.tensor.matmul(ps[:], h0T[:, 0:1], w1sb[i][:, c0:c0 + GW],
                                 start=False, stop=False)
                nc.tensor.matmul(ps[:], h0T[:, 1:2], w1sb[4 + i][:, c0:c0 + GW],
                                 start=False, stop=True)
                dst = g1part[:, g * GW:(g + 1) * GW]
                if g % 2:
                    nc.scalar.activation(dst, ps[:], Act.Copy)
                else:
                    nc.vector.tensor_copy(dst, ps[:])
                if g % 4 == 3:  # stage the finished quarter while later groups run
                    q = g // 4
                    nc.sync.dma_start(cc_in_v[:, q * Q:(q + 1) * Q],
                                      g1part[:, q * Q:(q + 1) * Q])

            # ---- the single collective: AllGather of the partial gates ----
            # (cheaper as a first CC op than AllReduce; the 8-way sum runs on
            # the vector engine in fp32, which is also more accurate than the
            # CC cores' bf16 tree).  The input was staged in 4 chunks above,
            # pipelined behind the drains.
            nc.gpsimd.collective_compute(
                "AllGather", mybir.AluOpType.bypass,
                ins=[cc_in.opt()], outs=[cc_out.rearrange("(n) -> n").opt()],
                replica_groups=rg,
            )
            # readback rank-outer (96B contiguous runs per rank chunk), split
            # across both HWDGE queues (sync + scalar) to overlap dispatch
            g8 = sp.tile([128, NCORES * 48], DTW, tag="g8")
            cc_v = cc_out.rearrange("(r q t) -> q r t", r=NCORES, t=48)
            nc.sync.dma_start(g8[:, 0:192], cc_v[:, 0:4])
            nc.scalar.dma_start(g8[:, 192:384], cc_v[:, 4:8])

            # ---- post-AllGather: 8-way tree sum, contiguous reads ----
            h4 = ap.tile([128, 4 * 48], DT, tag="h4")
            nc.vector.tensor_add(h4[:], g8[:, 0:192], g8[:, 192:384])
            h2 = ap.tile([128, 2 * 48], DT, tag="h2")
            nc.vector.tensor_add(h2[:], h4[:, 0:96], h4[:, 96:192])
            gsum = ap.tile([128, 48], DT, tag="gsum")
            nc.vector.tensor_add(gsum[:], h2[:, 0:48], h2[:, 48:96])
            t_all = ap.tile([128, 48], DT, tag="t_all")
            nc.scalar.activation(t_all[:], gsum[:], Act.Tanh, scale=0.5)
            t_io1 = t_all  # cols 0:16 = tanh(i/2), 16:32 = tanh(o/2)
            t_g1 = t_all[:, 32:48]  # g rows were pre-doubled: tanh(2g/2) = tanh(g)
            tmpa = ap.tile([128, 16], DT, tag="tmpa")
            nc.vector.tensor_mul(tmpa[:], t_g1, t_io1[:, 0:16])
            c21 = ap.tile([128, 16], DT, tag="c21")
            nc.vector.tensor_add(c21[:], tmpa[:], t_g1)
            t_c1 = ap.tile([128, 16], DT, tag="t_c1")
            nc.scalar.activation(t_c1[:], c21[:], Act.Tanh, scale=0.5)
            tmpb = ap.tile([128, 16], DT, tag="tmpb")
            nc.vector.tensor_mul(tmpb[:], t_c1[:], t_io1[:, 16:32])
            h1 = ap.tile([128, 16], DTW, tag="h1")
            nc.vector.tensor_add(h1[:], tmpb[:], t_c1[:])

            # ---- choice head: logits = 0.5*WC @ h1' + bc, computed locally ----
            psH = pp.tile([1, CH], DT, tag="psH")
            for u in range(16):
                nc.tensor.matmul(psH[:], h1[:, u:u + 1], wcsb[:, u * CH:(u + 1) * CH],
                                 start=(u == 0), stop=(u == 15))
            outsb = ap.tile([1, CH], DT, tag="outsb")
            nc.vector.tensor_add(outsb[:], psH[:], bcsb[:])
            nc.sync.dma_start(out.rearrange("(a n) -> a n", a=1), outsb[:])

    nc.compile()
    return nc


_NC_CACHE = None


def _get_nc():
    global _NC_CACHE
    if _NC_CACHE is None:
        _NC_CACHE = _build_nc()
    return _NC_CACHE


# --------------------------------------------------------------------------
# entry point
# --------------------------------------------------------------------------

def kernel(**inputs) -> np.ndarray:
    task = int(np.asarray(inputs["task"]).reshape(-1)[0]) if not isinstance(
        inputs["task"], int) else int(inputs["task"])
    maps = _host_prep(inputs)
    nc = _get_nc()
    for attempt in range(3):
        res = run_bass_kernel_spmd(nc, maps, list(range(NCORES)))
        outs = [np.asarray(res.results[i]["out"], np.float32).reshape(CH)
                for i in range(NCORES)]
        # post-AllReduce every core holds the same logits (up to reduction
        #-order LSBs); gross disagreement means a bad device state -- retry
        if all(np.allclose(outs[0], o, atol=1e-3) for o in outs[1:]):
            break
    logits = outs[0]
    mask = np.arange(CH) < (1 + task)
    return np.where(mask, logits, np.float32(-1e9)).astype(np.float32)


if __name__ == "__main__":
    import reference  # only for standalone debugging; not used by the grader

    inputs = reference.setup_inputs()
    expected = np.asarray(reference.reference(**inputs))
    actual = kernel(**inputs)
    print("expected:", expected)
    print("actual:  ", actual)
    denom = np.abs(expected).max()
    print("max abs err:", np.abs(actual - expected).max(),
          "rel:", np.abs(actual - expected).max() / denom)
